# revision 27
# baseline (speedup 1.0000x reference)
"""Trainium2 Bass kernel for nn_AttentionBranch: conv->relu->maxpool->conv->relu
followed by per-location rank-1 Gram outer products (100, 1024, 1024).

Sharding: the 100-location Gram axis is split across 8 NeuronCores
(13/12 locations per core). The conv1 backbone is fully REPLICATED on
every core (bf16) so no collective is needed at all. conv2 is
channel-sliced to each core's needed 136-channel window. The row-major
.view(100, 1024) of the conv2 output is realised through a small bf16
DRAM scratch roundtrip.

Output: per location only the upper "staircase" of 36 of the 64
128x128 blocks of the symmetric Gram matrix is computed and stored
(row-block r keeps columns [128r, 1024)), one contiguous 9216B/partition
store per location; the host reconstructs the lower blocks by
transposition.

Perf notes: PE is pre-warmed with dummy matmuls during the input DMA
(and across the conv2->gram transition) so the HAM clock gate stays at
2.4GHz. conv1 runs in two m-chunk waves so conv2's accumulation and
the activations/pools overlap conv1's second wave. The Gram stage
broadcasts each row via a K=2 bf16 matmul, evacuates PSUM->SBUF bf16 on
GPSIMD, then runs the staircase tensor_scalar ops from SBUF where DVE
gets the 4x 16-bit packed mode. Stores alternate the two HWDGE queues
and saturate HBM writes.
"""
import os
import numpy as np

# per-core location starts (each core computes 13 consecutive locations;
# odd cores' first location duplicates the previous core's last)
_LO = [0, 12, 25, 37, 50, 62, 75, 87]
# conv2 channel-slice starts; delta_k = 1024*lo_k - 100*ch_lo_k is 0 (even k)
# or 88 (odd k)
_CH_LO = [0, 122, 256, 378, 512, 634, 768, 890]
_NSL = 136  # channels per conv2 slice (covers 88 + 13*1024 flat elements)
# staircase column offsets: block r (width 1024-128r) starts at _OFF[r]
_OFF = [0, 1024, 1920, 2688, 3328, 3840, 4224, 4480]

_WARMUP = 52     # PE warmup matmuls (keep HAM at 2.4GHz through load phase)
_WARMUP_N = 160
_GAPWARM = 22    # PE keep-warm matmuls across the conv2->gram DMA gap

_CACHE = {}


def _build_nc():
    from concourse import bacc, tile, mybir

    f32 = mybir.dt.float32
    f16 = mybir.dt.float16
    bf16 = mybir.dt.bfloat16
    AF = mybir.ActivationFunctionType

    nc = bacc.Bacc("TRN2", target_bir_lowering=False, debug=False)

    inp_d = nc.dram_tensor("inp", [128, 4, 27, 25], bf16, kind="ExternalInput")
    w1_d = nc.dram_tensor("w1t", [128, 4, 9, 512], bf16, kind="ExternalInput")
    b1_d = nc.dram_tensor("b1t", [128, 4], f32, kind="ExternalInput")
    w2_d = nc.dram_tensor("w2t", [128, 4, 9, _NSL], bf16, kind="ExternalInput")
    b2_d = nc.dram_tensor("b2t", [128, 2], f32, kind="ExternalInput")
    selw_d = nc.dram_tensor("selw", [2, 128], bf16, kind="ExternalInput")
    selid_d = nc.dram_tensor("selid", [26, 16], bf16, kind="ExternalInput")
    g_d = nc.dram_tensor("gstair", [13, 128, 4608], f16, kind="ExternalOutput")
    scr_d = nc.dram_tensor("scratch", [137, 100], bf16)

    with tile.TileContext(nc) as tc:
        with tc.tile_pool(name="consts", bufs=1) as cp, \
             tc.tile_pool(name="work", bufs=1) as wp:

            w2sb = cp.tile([128, 4, 9, _NSL], bf16)
            b1sb = cp.tile([128, 4], f32)
            b2sb = cp.tile([128, 2], f32)
            selwsb = cp.tile([2, 128], bf16)
            selidsb = cp.tile([26, 16], bf16)

            convp = tc.alloc_tile_pool(name="convp", bufs=1)
            ps1 = tc.alloc_tile_pool(name="ps1", bufs=1, space="PSUM")
            ps2 = tc.alloc_tile_pool(name="ps2", bufs=2, space="PSUM")
            psC = tc.alloc_tile_pool(name="psC", bufs=1, space="PSUM")
            insb = convp.tile([128, 4, 27, 25], bf16)
            w1sb = [convp.tile([128, 9, 512], bf16, name=f"w1c{c}")
                    for c in range(4)]

            # critical loads first, alternating the two HWDGE queues
            nc.sync.dma_start(out=insb[:], in_=inp_d.ap())
            nc.scalar.dma_start(out=w1sb[0][:], in_=w1_d.ap()[:, 0])
            nc.sync.dma_start(out=w1sb[1][:], in_=w1_d.ap()[:, 1])
            nc.scalar.dma_start(out=w1sb[2][:], in_=w1_d.ap()[:, 2])
            nc.sync.dma_start(out=w1sb[3][:], in_=w1_d.ap()[:, 3])
            nc.scalar.dma_start(out=w2sb[:], in_=w2_d.ap())
            nc.scalar.dma_start(out=b1sb[:], in_=b1_d.ap())
            nc.scalar.dma_start(out=b2sb[:], in_=b2_d.ap())
            nc.scalar.dma_start(out=selwsb[:], in_=selw_d.ap())
            nc.scalar.dma_start(out=selidsb[:], in_=selid_d.ap())

            # conv1 psum: 4 banks, reused across the two m-chunk waves
            GB = [(0, 12, 300), (12, 11, 275)]  # (row0, nrows, N)

            def wave_tiles():
                return [[ps1.tile([128, GB[g][2]], f32, tag=f"c1p{j}_{g}",
                                  name=f"c1p{j}_{g}")
                         for g in range(2)] for j in range(2)]

            c1sb = wp.tile([128, 4, 24, 24], f32)
            nc.vector.memset(c1sb[:, :, 23:24, :], 0.0)
            nc.vector.memset(c1sb[:, :, :, 23:24], 0.0)

            pc = psC.tile([128, 8, 16], f32)
            pcflat = pc[:].rearrange("p a b -> p (a b)")

            # PE warmup: junk matmuls with no DMA deps keep the PE busy from
            # preamble end until conv1's inputs land, so HAM is at 8/8
            wamm = wp.tile([128, 128], bf16)
            nc.vector.memset(wamm[:], 0.0)
            T2h = wp.tile([26, 1024], bf16)
            nc.vector.memset(T2h[:], 0.0)
            for i in range(_WARMUP):
                nc.tensor.matmul(pcflat[:, 0:128], wamm[:],
                                 wamm[:], start=True, stop=True)

            # ---- conv1 FULL (512 channels), bf16, two waves of m-chunks;
            # c-outer so matmuls start as soon as ci-chunk 0 lands ----
            def conv1_wave(wave, tiles):
                for c in range(4):
                    flat_c = insb[:, c].rearrange("p a b -> p (a b)")
                    for j in range(2):
                        m = 2 * wave + j
                        for t in range(9):
                            dy, dx = t // 3, t % 3
                            for g, (r0, nr, N) in enumerate(GB):
                                s0 = (r0 + dy) * 25 + dx
                                nc.tensor.matmul(
                                    tiles[j][g][:, :],
                                    w1sb[c][:, t, 128 * m:128 * m + 128],
                                    flat_c[:, s0:s0 + N],
                                    start=(c == 0 and t == 0),
                                    stop=(c == 3 and t == 8),
                                )

            colmax = wp.tile([128, 4, 24, 12], f32)
            pooled = wp.tile([128, 4, 12, 12], bf16)

            def act_pool(m, tiles):
                j = m % 2
                for g, (r0, nr, N) in enumerate(GB):
                    nc.scalar.activation(
                        out=c1sb[:, m, r0:r0 + nr, 0:23],
                        in_=tiles[j][g][:, :].rearrange(
                            "p (a b) -> p a b", b=25)[:, 0:nr, 0:23],
                        func=AF.Relu,
                        bias=b1sb[:, m:m + 1],
                    )
                cpair = c1sb[:, m].rearrange("p r (w two) -> p r w two", two=2)
                nc.vector.tensor_max(colmax[:, m], cpair[:, :, :, 0],
                                     cpair[:, :, :, 1])
                rpair = colmax[:, m].rearrange("p (r two) w -> p r two w",
                                               two=2)
                nc.vector.tensor_max(pooled[:, m], rpair[:, :, 0, :],
                                     rpair[:, :, 1, :])

            tiles_a = wave_tiles()
            conv1_wave(0, tiles_a)
            act_pool(0, tiles_a)
            act_pool(1, tiles_a)

            # wave 2 reuses the same psum banks (WAR-tracked); conv2's
            # early ci-chunks overlap it on the PE stream
            tiles_b = wave_tiles()
            conv1_wave(1, tiles_b)
            act_pool(2, tiles_b)
            act_pool(3, tiles_b)

            # ---- conv2 slice: 136 output channels, bf16; the small m=1
            # chunk goes FIRST so after the m=0 scratch store every flat
            # load below is immediately ready ----
            c2sb = wp.tile([128, 2, 100], bf16)
            for mo, mw, sl in ((128, 8, 1), (0, 128, 0)):
                ps = ps2.tile([128, 100], f32, tag="c2p")
                for c in range(4):
                    for t in range(9):
                        dy, dx = t // 3, t % 3
                        nc.tensor.matmul(
                            ps[0:mw, :],
                            w2sb[:, c, t, mo:mo + mw],
                            pooled[:, c, dy:dy + 10, dx:dx + 10],
                            start=(c == 0 and t == 0),
                            stop=(c == 3 and t == 8),
                        )
                nc.scalar.activation(
                    out=c2sb[0:mw, sl, :],
                    in_=ps[0:mw, :],
                    func=AF.Relu,
                    bias=b2sb[0:mw, sl:sl + 1],
                )
                if sl == 0:
                    # split so the first flat loads can chase the head
                    nc.sync.dma_start(out=scr_d.ap()[0:42, :],
                                      in_=c2sb[0:42, 0, :])
                    nc.sync.dma_start(out=scr_d.ap()[42:128, :],
                                      in_=c2sb[42:128, 0, :])
                else:
                    nc.scalar.dma_start(out=scr_d.ap()[128:136, :],
                                        in_=c2sb[0:8, 1, :])

            # keep the PE busy (and the HAM warm) across the scratch
            # roundtrip so ccol/bp matmuls run at 2.4GHz
            for i in range(_GAPWARM):
                nc.tensor.matmul(pcflat[:, 0:128], wamm[:, 0:128],
                                 wamm[:, 0:128], start=True, stop=True)

            # flat .view(13,1024) rows: T2 holds both delta variants for the
            # column factors; t2row holds them as 2 long rows for the PE
            # row-broadcast. Same-queue ordering after the m=0 store avoids
            # an extra cross-queue rendezvous for the sync-queue loads.
            flat = scr_d.ap().rearrange("a b -> (a b)")
            T2 = wp.tile([26, 1024], bf16)
            t2row = wp.tile([2, 13312], bf16)
            # head chunk (locs 0-3) depends only on scratch rows 0-41, so
            # the first Gram locations start while the rest still loads.
            # T2h is a separate tile (rows 4-12/17-25 stay zero) so the
            # head ccol matmuls don't wait on the full T2 loads.
            nc.sync.dma_start(out=t2row[0:1, 0:4096], in_=flat[0:4096])
            nc.scalar.dma_start(out=t2row[1:2, 0:4096], in_=flat[88:4184])
            nc.sync.dma_start(
                out=T2h[0:4, :],
                in_=flat[0:4096].rearrange("(p i) -> p i", i=1024))
            nc.scalar.dma_start(
                out=T2h[13:17, :],
                in_=flat[88:4184].rearrange("(p i) -> p i", i=1024))
            nc.sync.dma_start(out=t2row[0:1, 4096:13312],
                              in_=flat[4096:13312])
            nc.scalar.dma_start(out=t2row[1:2, 4096:13312],
                                in_=flat[4184:13400])
            nc.sync.dma_start(
                out=T2[0:13, :],
                in_=flat[0:13312].rearrange("(p i) -> p i", i=1024))
            nc.scalar.dma_start(
                out=T2[13:26, :],
                in_=flat[88:13400].rearrange("(p i) -> p i", i=1024))

            # column factors: ccol[p, r, l] = v_l[128r + p], selecting the
            # delta-0/delta-88 variant via the selid block-diagonal.
            # Split locs 0-3 / 4-12 so the head only waits on the first
            # flat chunk (T2 rows 4-12/17-25 are zeroed: garbage would
            # otherwise reach the head matmuls as NaN*0).
            ccol = wp.tile([128, 8, 16], f32)
            for r in range(8):
                nc.tensor.matmul(pc[:, r, 0:4], T2h[:, 128 * r:128 * r + 128],
                                 selidsb[0:26, 0:4], start=True, stop=True)
            nc.vector.tensor_copy(ccol[:, :, 0:4], pc[:, :, 0:4])
            for r in range(8):
                nc.tensor.matmul(pc[:, r, 4:13], T2[:, 128 * r:128 * r + 128],
                                 selidsb[0:26, 4:13], start=True, stop=True)
            nc.vector.tensor_copy(ccol[:, :, 4:13], pc[:, :, 4:13])

            psC.release()
            ps2.release()
            ps1.release()
            convp.release()
            psB = tc.alloc_tile_pool(name="psB", bufs=3, space="PSUM")
            vp = tc.alloc_tile_pool(name="bcast", bufs=3)
            sp = tc.alloc_tile_pool(name="stage", bufs=5)

            # ---- Gram staircase ----
            # DVE evacuates the low PSUM half and takes widths
            # {1024, 896, 768, 512, 384}; ACT evacuates the high half and
            # takes {640, 256, 128} (GPSIMD tensor ops are ~10x slower --
            # measured -- so it gets nothing here)
            ENG = {0: 'v', 1: 'v', 2: 'v', 3: 'a', 4: 'v',
                   5: 'v', 6: 'a', 7: 'a'}
            for li in range(13):
                bp = psB.tile([128, 1024], f32, tag="bc")
                nc.tensor.matmul(bp[:, 0:512], selwsb[:],
                                 t2row[:, 1024 * li:1024 * li + 512],
                                 start=True, stop=True)
                nc.tensor.matmul(bp[:, 512:1024], selwsb[:],
                                 t2row[:, 1024 * li + 512:1024 * (li + 1)],
                                 start=True, stop=True)
                bc = vp.tile([128, 1024], bf16, tag="bcs")
                nc.vector.tensor_copy(bc[:, 0:512], bp[:, 0:512])
                nc.scalar.activation(bc[:, 512:1024], bp[:, 512:1024],
                                     func=AF.Copy)
                st = sp.tile([128, 4608], f16, tag="st")
                for r in range(8):
                    w = 1024 - 128 * r
                    src = bc[:, 128 * r:1024]
                    dst = st[:, _OFF[r]:_OFF[r] + w]
                    col = ccol[:, r, li:li + 1]
                    if ENG[r] == 'a':
                        nc.scalar.activation(dst, src, func=AF.Copy,
                                             scale=col)
                    else:
                        nc.vector.tensor_scalar_mul(dst, src, col)
                if li == 12:
                    # split the final store across both queues to halve
                    # the last-drain tail
                    half = st[:].rearrange("p (h f) -> p h f", h=2)
                    gdst = g_d.ap()[li].rearrange("p (h f) -> p h f", h=2)
                    nc.sync.dma_start(out=gdst[:, 0], in_=half[:, 0])
                    nc.scalar.dma_start(out=gdst[:, 1], in_=half[:, 1])
                else:
                    eng = nc.sync if li % 2 == 0 else nc.scalar
                    eng.dma_start(out=g_d.ap()[li], in_=st[:])
            sp.release()
            vp.release()
            psB.release()

    nc.compile()
    return nc


def _get_nc():
    if "nc" not in _CACHE:
        _CACHE["nc"] = _build_nc()
    return _CACHE["nc"]


def _host_prep(input, w1, b1, w2, b2):
    import ml_dtypes
    bf = ml_dtypes.bfloat16

    x = np.asarray(input, np.float32).reshape(512, 25, 25)
    w1 = np.asarray(w1, np.float32)
    w2 = np.asarray(w2, np.float32)
    b1 = np.asarray(b1, np.float32)
    b2 = np.asarray(b2, np.float32)

    inp = np.zeros((4, 128, 27, 25), np.float32)
    inp[:, :, :25, :] = x.reshape(4, 128, 25, 25)
    inp = np.ascontiguousarray(inp.transpose(1, 0, 2, 3)).astype(bf)

    w1t = w1.reshape(512, 512, 9).transpose(1, 2, 0)          # [ci, 9, co]
    w1t = np.ascontiguousarray(
        w1t.reshape(4, 128, 9, 512).transpose(1, 0, 2, 3)).astype(bf)
    b1t = np.ascontiguousarray(b1.reshape(4, 128).T)          # [128, 4]

    common = {"inp": inp, "w1t": w1t, "b1t": b1t}
    in_maps = []
    for k in range(8):
        ch = _CH_LO[k]
        nval = min(1024, ch + _NSL) - ch
        wsl = np.zeros((_NSL, 512, 9), np.float32)
        wsl[:nval] = w2.reshape(1024, 512, 9)[ch:ch + nval]
        w2t = wsl.transpose(1, 2, 0)                           # [512,9,136]
        w2t = np.ascontiguousarray(
            w2t.reshape(4, 128, 9, _NSL).transpose(1, 0, 2, 3)).astype(bf)
        bsl = np.zeros(256, np.float32)
        bsl[:nval] = b2[ch:ch + nval]
        b2t = np.ascontiguousarray(bsl.reshape(2, 128).T)
        s0 = 1.0 if (1024 * _LO[k] - 100 * ch) == 0 else 0.0
        selw = np.zeros((2, 128), np.float32)
        selw[0, :] = s0
        selw[1, :] = 1.0 - s0
        selid = np.zeros((26, 16), np.float32)
        selid[0:13, 0:13] = s0 * np.eye(13, dtype=np.float32)
        selid[13:26, 0:13] = (1.0 - s0) * np.eye(13, dtype=np.float32)
        in_maps.append({**common, "w2t": w2t, "b2t": b2t,
                        "selw": selw.astype(bf), "selid": selid.astype(bf)})
    return in_maps


def kernel(input, w1, b1, w2, b2):
    from concourse import bass_utils

    nc = _get_nc()
    in_maps = _host_prep(input, w1, b1, w2, b2)

    prof_dir = os.environ.get("GRAM_KERNEL_PROFILE_DIR")
    if prof_dir:
        from trn_agent_boot.trn_boot import _ntff_profile_via_ctypes
        hook = _ntff_profile_via_ctypes('/opt/axon/libaxon_pjrt.so')
        with hook(prof_dir, [0]):
            res = bass_utils.run_bass_kernel_spmd(
                nc, in_maps, core_ids=list(range(8)))
    else:
        res = bass_utils.run_bass_kernel_spmd(
            nc, in_maps, core_ids=list(range(8)))

    out = np.empty((100, 1024, 1024), np.float32)
    for k in range(8):
        S = np.asarray(res.results[k]["gstair"])   # [13, 128, 4608] f16
        j0 = k % 2   # odd cores' first row duplicates previous core's last
        lo = _LO[k]
        for r in range(8):
            w = 1024 - 128 * r
            out[lo + j0:lo + 13, 128 * r:128 * r + 128, 128 * r:1024] = \
                S[j0:13, :, _OFF[r]:_OFF[r] + w]
    # lower blocks are transposes of the stored upper staircase
    for R in range(1, 8):
        for C in range(R):
            out[:, 128 * R:128 * R + 128, 128 * C:128 * C + 128] = \
                out[:, 128 * C:128 * C + 128,
                    128 * R:128 * R + 128].transpose(0, 2, 1)
    return out
